# revision 5
# baseline (speedup 1.0000x reference)
"""BitLinear (BitNet b1.58) Trainium2 kernel, 8-core data-parallel.

Reference computation (fp32):
    scale  = 127 / clip(max|x| over d_in, 1e-5)          (per token)
    xq     = clip(round(x*scale), -128, 127) / scale     (per-token int8 quant-dequant)
    s      = clip(mean|W|, 1e-5)
    wq     = clip(round(W/s), -1, 1) * s                 (ternary quant)
    out    = xq @ wq.T

Kernel strategy (per core, tokens sharded 4096/core, weight replicated):
    q  = round(x*scale)  are integers in [-127,127]  -> exact in bf16
    t  = clip(round(W/s),-1,1) in {-1,0,1}           -> exact in bf16
    q @ t.T accumulated in fp32 PSUM is EXACT integer arithmetic, then
    out = psum * (absmax * s / 127) per token, stored bf16.

    v2 vs v1: the weight is fed host-pre-transposed (wT = W.T, a layout/
    sharding choice) so ternary quantization happens in-place with no PE
    transposes; the activation transpose q -> qT is done by the DMA XBAR
    (dma_start_transpose, bf16) instead of PE identity-matmuls.  The PE
    runs ONLY the 512 main bf16 matmuls (~110us), everything else hides
    behind them.  Rounding uses the fp32 magic-number trick
    round(v) = (v + 1.5*2^23) - 1.5*2^23  (round-half-even, matches
    jnp.round bitwise).
"""

import numpy as np

import concourse.bass as bass
import concourse.mybir as mybir
from concourse import tile
from concourse.bass_utils import run_bass_kernel_spmd

F32 = mybir.dt.float32
BF16 = mybir.dt.bfloat16

N_CORES = 8
B, S, D_IN, D_OUT = 4, 8192, 1024, 1024
TOKENS = B * S                     # 32768
TOK_PER_CORE = TOKENS // N_CORES   # 4096
TILES = TOK_PER_CORE // 128        # 32
KT = D_IN // 128                   # 8 contraction k-chunks

QMAX = 127.0
MAGIC = 12582912.0                     # 1.5 * 2**23 -> RNE integer rounding
THR = float(np.nextafter(np.float32(1.5), np.float32(0)))  # largest f32 < 1.5


def _split_multiwaits(nc):
    """walrus here encodes at most ONE sem wait per instruction; Tile's tail
    drain (and occasionally other insts) carry several.  Split extras into
    single-wait NOPs on the same engine, preserving order."""
    for f in nc.m.functions:
        for bb in f.blocks:
            insts = list(bb.instructions)
            if not any(
                i.sync_info and len(i.sync_info.on_wait) > 1 for i in insts
            ):
                continue
            new = []
            for ins in insts:
                si = ins.sync_info
                if si and len(si.on_wait) > 1:
                    waits = list(si.on_wait)
                    for j, w in enumerate(waits[:-1]):
                        nop = mybir.InstNoOp(
                            name=f"{ins.name}_wsp{j}", ins=[], outs=[]
                        )
                        nop.engine = ins.engine
                        nop.sync_info = mybir.SyncInfo(on_wait=[w], on_update=[])
                        new.append(nop)
                    ins.sync_info = mybir.SyncInfo(
                        on_wait=[waits[-1]], on_update=list(si.on_update)
                    )
                new.append(ins)
            bb.instructions = new


def build_program():
    nc = bass.Bass(trn_type="TRN2")
    x_d = nc.dram_tensor("x", [TOK_PER_CORE, D_IN], F32, kind="ExternalInput")
    w_d = nc.dram_tensor("wT", [D_IN, D_OUT], F32, kind="ExternalInput")
    o_d = nc.dram_tensor("out", [TOK_PER_CORE, D_OUT], BF16, kind="ExternalOutput")

    Copy = mybir.ActivationFunctionType.Copy
    Abs = mybir.ActivationFunctionType.Abs
    AX = mybir.AxisListType.X
    op = mybir.AluOpType

    with tile.TileContext(nc) as tc:
        from contextlib import ExitStack

        with ExitStack() as ctx:
            singles = ctx.enter_context(tc.tile_pool(name="singles", bufs=1))

            ones_col = singles.tile([128, 1], F32)
            nc.vector.memset(ones_col[:], 1.0)
            ones_row = singles.tile([1, 128], F32)
            nc.vector.memset(ones_row[:], 1.0)
            bc2 = singles.tile([128, 2], F32)    # [s, 1/s] broadcast to 128 parts
            s127_bc = singles.tile([128, 1], F32)  # s/127 broadcast

            tT = [singles.tile([128, D_OUT], BF16, name=f"tT{k}", tag=f"tT{k}") for k in range(KT)]

            xpool = ctx.enter_context(tc.tile_pool(name="xpool", bufs=7))
            xmpool = ctx.enter_context(tc.tile_pool(name="xmpool", bufs=3))
            qpool = ctx.enter_context(tc.tile_pool(name="qpool", bufs=3))
            qtpool = ctx.enter_context(tc.tile_pool(name="qtpool", bufs=4))
            outpool = ctx.enter_context(tc.tile_pool(name="outpool", bufs=3))
            smpool = ctx.enter_context(tc.tile_pool(name="smpool", bufs=10))
            pso = ctx.enter_context(tc.tile_pool(name="pso", bufs=4, space="PSUM"))
            psm = ctx.enter_context(tc.tile_pool(name="psm", bufs=2, space="PSUM"))

            live = {}

            def a_dma(n):
                """x tile DMA + per-token absmax/scale (DVE)."""
                x_t = xpool.tile([128, D_IN], F32, tag="x")
                nc.sync.dma_start(x_t[:], x_d[n * 128:(n + 1) * 128, :])
                am = smpool.tile([128, 1], F32, tag="am")
                nc.vector.tensor_reduce(
                    am[:], x_t[:], axis=AX, op=op.max, apply_absolute_value=True
                )
                ram = smpool.tile([128, 1], F32, tag="ram")
                nc.vector.reciprocal(ram[:], am[:])
                scl = smpool.tile([128, 1], F32, tag="scl")
                nc.vector.tensor_scalar(scl[:], ram[:], QMAX, None, op0=op.mult)
                live[("x", n)] = x_t
                live[("am", n)] = am
                live[("scl", n)] = scl

            def a_quant(n):
                """quantize to bf16 integers (ACT+DVE), transpose via DMA XBAR."""
                x_t = live.pop(("x", n))
                scl = live.pop(("scl", n))
                xm = xmpool.tile([128, D_IN], F32, tag="xm")
                nc.scalar.activation(xm[:], x_t[:], Copy, bias=MAGIC, scale=scl[:])
                q = qpool.tile([128, D_IN], BF16, tag="q")
                nc.vector.tensor_scalar(q[:], xm[:], -MAGIC, None, op0=op.add)
                qt = qtpool.tile([128, D_IN], BF16, tag="qt")
                qt3 = qt[:, :].rearrange("p (k t) -> p k t", k=KT)
                nc.sync.dma_start_transpose(qt3, q[:, :])
                live[("qt", n)] = qt

            def b(n):
                """16 matmuls (PE) + per-token rescale (ACT/DVE) + out DMA."""
                qt = live.pop(("qt", n))
                am = live.pop(("am", n))
                coef = smpool.tile([128, 1], F32, tag="coef")
                nc.vector.tensor_scalar(coef[:], am[:], s127_bc[:], None, op0=op.mult)
                ps_a = pso.tile([128, 512], F32, tag="ps")
                ps_b = pso.tile([128, 512], F32, tag="ps")
                for k in range(KT):
                    lhsT = qt[:, k * 128:(k + 1) * 128]
                    nc.tensor.matmul(
                        ps_a[:], lhsT, tT[k][:, 0:512],
                        start=(k == 0), stop=(k == KT - 1),
                    )
                    nc.tensor.matmul(
                        ps_b[:], lhsT, tT[k][:, 512:1024],
                        start=(k == 0), stop=(k == KT - 1),
                    )
                out_sb = outpool.tile([128, D_OUT], BF16, tag="osb")
                nc.scalar.activation(out_sb[:, 0:512], ps_a[:], Copy, scale=coef[:])
                nc.vector.tensor_scalar(
                    out_sb[:, 512:1024], ps_b[:], coef[:], None, op0=op.mult
                )
                nc.scalar.dma_start(o_d[n * 128:(n + 1) * 128, :], out_sb[:])

            # ---------------- weight phase (interleaved with x ramp) -------
            with (
                tc.tile_pool(name="wpool", bufs=1) as wpool,
                tc.tile_pool(name="wabs", bufs=2) as wabs_pool,
                tc.tile_pool(name="ypool", bufs=2) as ypool,
            ):
                # DMA order on the SP ring sets HBM priority: weight chunks
                # first (they gate the matmuls), early x tiles interleaved.
                w_t = [wpool.tile([128, D_OUT], F32, name=f"w{k}", tag=f"w{k}") for k in range(KT)]
                nc.sync.dma_start(w_t[0][:], w_d[0:128, :])
                nc.sync.dma_start(w_t[1][:], w_d[128:256, :])
                a_dma(0)
                nc.sync.dma_start(w_t[2][:], w_d[256:384, :])
                a_dma(1)
                nc.sync.dma_start(w_t[3][:], w_d[384:512, :])
                a_dma(2)
                nc.sync.dma_start(w_t[4][:], w_d[512:640, :])
                a_dma(3)
                for k in range(5, KT):
                    nc.sync.dma_start(w_t[k][:], w_d[k * 128:(k + 1) * 128, :])
                a_dma(4)
                a_dma(5)

                # |w| row-chunk sums: split ACT (Abs+accum) / DVE (abs-reduce)
                colsum = wpool.tile([128, KT], F32)
                for k in range(KT):
                    if k % 2 == 0:
                        wabs = wabs_pool.tile([128, D_OUT], F32, tag="wabs")
                        nc.scalar.activation(
                            wabs[:], w_t[k][:], Abs, accum_out=colsum[:, k:k + 1]
                        )
                    else:
                        nc.vector.tensor_reduce(
                            colsum[:, k:k + 1], w_t[k][:], axis=AX, op=op.add,
                            apply_absolute_value=True,
                        )
                colsum2 = wpool.tile([128, 1], F32)
                nc.vector.tensor_reduce(colsum2[:], colsum[:], axis=AX, op=op.add)

                ps_m1 = psm.tile([1, 2], F32, name="ps_m1", tag="ps_m")
                nc.tensor.matmul(ps_m1[0:1, 0:1], ones_col[:], colsum2[:])
                pair = wpool.tile([1, 2], F32)
                nc.scalar.activation(pair[:, 0:1], ps_m1[0:1, 0:1], Copy, scale=1.0 / (D_OUT * D_IN))
                nc.vector.reciprocal(pair[:, 1:2], pair[:, 0:1])
                ps_m2 = psm.tile([128, 2], F32, name="ps_m2", tag="ps_m")
                nc.tensor.matmul(ps_m2[:], ones_row[:], pair[:])
                nc.scalar.copy(bc2[:], ps_m2[:])
                nc.vector.tensor_scalar(s127_bc[:], bc2[:, 0:1], 1.0 / QMAX, None, op0=op.mult)

                # ternary quantize wT chunks in place (no PE involvement);
                # interleave remaining ramp-tile quant work
                a_quant(0)
                for k in range(KT):
                    y0 = ypool.tile([128, D_OUT], F32, tag="y0")
                    nc.scalar.activation(y0[:], w_t[k][:], Copy, scale=bc2[:, 1:2])
                    y1 = ypool.tile([128, D_OUT], F32, tag="y1")
                    nc.vector.tensor_scalar(y1[:], y0[:], THR, -THR, op0=op.min, op1=op.max)
                    nc.vector.tensor_scalar(
                        tT[k][:], y1[:], MAGIC, -MAGIC, op0=op.add, op1=op.add
                    )
                    if k == 1:
                        a_quant(1)
                    elif k == 3:
                        a_quant(2)
                    elif k == 5:
                        a_quant(3)
                    elif k == 7:
                        a_quant(4)

            for n in range(TILES):
                b(n)
                if n + 6 < TILES:
                    a_dma(n + 6)
                if n + 5 < TILES:
                    a_quant(n + 5)

    _split_multiwaits(nc)
    return nc


_NC_CACHE = None


def _get_nc():
    global _NC_CACHE
    if _NC_CACHE is None:
        _NC_CACHE = build_program()
    return _NC_CACHE


def kernel(x: np.ndarray, weight: np.ndarray, trace: bool = False):
    assert x.shape == (B, S, D_IN) and weight.shape == (D_OUT, D_IN)
    nc = _get_nc()
    xf = np.ascontiguousarray(x.reshape(TOKENS, D_IN), dtype=np.float32)
    wT = np.ascontiguousarray(weight.astype(np.float32, copy=False).T)
    in_maps = [
        {
            "x": xf[c * TOK_PER_CORE:(c + 1) * TOK_PER_CORE],
            "wT": wT,
        }
        for c in range(N_CORES)
    ]
    res = run_bass_kernel_spmd(nc, in_maps, core_ids=list(range(N_CORES)), trace=trace)
    kernel.last_results = res
    out = np.concatenate(
        [np.asarray(res.results[c]["out"]).astype(np.float32) for c in range(N_CORES)],
        axis=0,
    )
    return out.reshape(B, S, D_OUT)


kernel.last_results = None
